# revision 46
# baseline (speedup 1.0000x reference)
"""Causal GQA attention block (RoPE, 16 q-heads / 4 kv-heads, D=1024, S=2048, B=2)
distributed over 8 NeuronCores: data-parallel over batch (2) x tensor-parallel
over kv-groups (4). Each core computes 4 query heads + 1 kv head for one batch
element, Megatron-style: Wq/Wk/Wv column-parallel, Wo row-parallel with the
row-parallel partial sums reduced on host.

v6 scheduling notes (on top of the v2 layout):
 - The input stream (~5.4MB) runs at ~210GB/s real => ~25us DMA wall; the
   whole schedule hugs it: DMA strictly by first need, x packed 4-chunks-
   per-transfer (4KB lines), cos/sin tables halved (de-duplicated), PE
   warmup matmuls ramp the DVFS p-state while the first megabyte streams.
 - Attention starts right after proj(0); proj(1) becomes filler inside
   attn(0). Every proj/Wo is chopped into ~213-426ns units trickled one per
   attention piece (the Exp on Act runs ~185ns/piece longer than the
   piece's PE work) and in ~2.4us lumps at the softmax-normalize (DVE
   rec+mul) boundaries where the av PSUM bank is held.
 - All proj groups emit stop->copy->rotate staggered one 4-matmul half
   behind, so the PE never waits on an Act/DVE copy; prologue rot psums
   borrow the idle av bank.
 - Causal mask is accumulated on the PE itself (trineg lhsT x identity-pair
   rhs into the score PSUM) - no cross-engine mask hop.
 - wo(3) splits around attn(3)'s h-boundary: the h0 half runs as mid-tile
   filler, the h1 half is the only tail work after the last reciprocal.
"""

import os
import sys
import types
from collections import deque

import numpy as np

import concourse.bass as bass
import concourse.mybir as mybir
import concourse.tile as tile
from concourse import bacc
from concourse.bass_utils import run_bass_kernel_spmd

F32 = mybir.dt.float32
BF16 = mybir.dt.bfloat16
AF = mybir.ActivationFunctionType

B, S, D = 2, 2048, 1024
H, KV, HD = 16, 4, 64
NH = 4  # query heads per core
P = 128
NT = S // 512  # 4 i-tiles of 512
KC = D // P  # 8 contraction chunks
JC = S // P  # 16 j-chunks
N_CORES = 8
N_WARM = 12  # PE warmup matmuls (DVFS ramp) while the DMA prologue streams

_cached = {}


def _install_trace_hook():
    """NTFF profiling hook shim (the container's antenv lacks axon_hooks)."""
    try:
        import antenv

        if "antenv.axon_hooks" in sys.modules:
            return
        mod = types.ModuleType("antenv.axon_hooks")
        _h = [None]
        mod.set_axon_ntff_profile_hook = lambda h: _h.__setitem__(0, h)
        mod.get_axon_ntff_profile_hook = lambda: _h[0]
        sys.modules["antenv.axon_hooks"] = mod
        antenv.axon_hooks = mod
        from trn_agent_boot.trn_boot import _ntff_profile_via_ctypes

        mod.set_axon_ntff_profile_hook(
            _ntff_profile_via_ctypes("/opt/axon/libaxon_pjrt.so")
        )
    except Exception:
        pass


def build_bass():
    nc = bacc.Bacc("TRN2", target_bir_lowering=False, debug=False, num_devices=N_CORES)

    # x4[g*4+b] = fused cols [512b, 512b+512) of chunks 4g..4g+3: 4KB DMA
    # lines per partition (1KB lines ran at ~half the DMA roofline)
    x4 = nc.dram_tensor("x4", [8, P, 4 * 512], BF16, kind="ExternalInput")
    # host pre-packed partition-major: one contiguous line per partition
    wq = nc.dram_tensor("wq", [P, KC * NH * HD], BF16, kind="ExternalInput")
    wkv = nc.dram_tensor("wkv", [P, KC * 2 * HD], BF16, kind="ExternalInput")
    wo = nc.dram_tensor("wo", [P, 2 * D], BF16, kind="ExternalInput")
    cos2 = nc.dram_tensor("cos2", [HD, S], BF16, kind="ExternalInput")
    sin2 = nc.dram_tensor("sin2", [HD, S], BF16, kind="ExternalInput")
    r2t = nc.dram_tensor("r2t", [P, P], BF16, kind="ExternalInput")
    # causal mask as PE accumulate: sc[j,(w,i)] += trineg[i,j] via identity
    # rhs pair; trineg[k,m] = -1e9 where m>k else 0
    trineg = nc.dram_tensor("trineg", [P, P], BF16, kind="ExternalInput")
    i2 = nc.dram_tensor("i2", [P, 2 * P], BF16, kind="ExternalInput")
    out = nc.dram_tensor("out", [S, D], BF16, kind="ExternalOutput")

    with tile.TileContext(nc) as tc:
        with (
            tc.tile_pool(name="const", bufs=1) as const,
            tc.tile_pool(name="persist", bufs=1) as persist,
            tc.tile_pool(name="sb_tmp", bufs=4) as sb_tmp,
            tc.tile_pool(name="sb_pt", bufs=8) as sb_pt,
            tc.tile_pool(name="sb_ot", bufs=4) as sb_ot,
            tc.tile_pool(name="sb_out", bufs=4) as sb_out,
            tc.tile_pool(name="ps", bufs=1, space="PSUM") as ps,
        ):
            # ---- PE warmup: memset tile -> matmul stream (ramps p-state) ----
            warm_sb = const.tile([P, 512], BF16)
            nc.gpsimd.memset(warm_sb[:], 0.0)
            for _ in range(N_WARM):
                wps = ps.tile([P, 512], F32, tag="bg", bufs=2, name="warm_ps")
                nc.tensor.matmul(
                    wps[:], warm_sb[:, 0:P], warm_sb[:], start=True, stop=True
                )

            # ---- DMA prologue: strict first-need order on 2 queues ----
            # xT lives as 2 fused tiles: chunk k = xT4_sb[k//4][:, k%4, :]
            xT4_sb = [
                persist.tile([P, 4, S], BF16, tag=f"xT4_{g}", name=f"xT4_sb{g}")
                for g in range(2)
            ]
            wkv_sb = const.tile([P, KC, 2 * HD], BF16)
            # DMA'd de-duplicated into rows 0:64, then shift-copied on DVE
            # (tensor_tensor needs equal SBUF base partitions; copy doesn't)
            cos_sb = const.tile([P, S], BF16)
            sin_sb = const.tile([P, S], BF16)
            r2t_sb = const.tile([P, P], BF16)
            wq_sb = const.tile([P, KC, NH * HD], BF16)
            trineg_sb = const.tile([P, P], BF16)
            i2_sb = const.tile([P, 2, P], BF16)
            wo_sb = const.tile([P, 2, D], BF16)

            def dma_x4(eng, g, b):
                eng.dma_start(
                    xT4_sb[g][:, :, b * 512 : (b + 1) * 512],
                    x4[g * 4 + b].rearrange("p (j c) -> p j c", j=4),
                )

            nc.sync.dma_start(wkv_sb[:], wkv.rearrange("p (k m) -> p k m", k=KC))
            dma_x4(nc.scalar, 0, 0)
            nc.scalar.dma_start(wq_sb[:], wq.rearrange("p (k m) -> p k m", k=KC))
            dma_x4(nc.sync, 1, 0)
            nc.sync.dma_start(r2t_sb[:], r2t[:])
            nc.scalar.dma_start(trineg_sb[:], trineg[:])
            nc.scalar.dma_start(i2_sb[:], i2.rearrange("p (c m) -> p c m", c=2))
            nc.sync.dma_start(cos_sb[0:HD, :], cos2[:])
            nc.scalar.dma_start(sin_sb[0:HD, :], sin2[:])
            # duplicate the de-duplicated tables into rows 64:128 on the
            # (idle) Pool engine - keeps DVE free for the rope chain
            nc.gpsimd.tensor_copy(cos_sb[HD:P, :], cos_sb[0:HD, :])
            nc.gpsimd.tensor_copy(sin_sb[HD:P, :], sin_sb[0:HD, :])
            dma_x4(nc.scalar, 0, 1)
            dma_x4(nc.sync, 1, 1)
            dma_x4(nc.sync, 0, 2)
            dma_x4(nc.scalar, 1, 2)
            dma_x4(nc.sync, 0, 3)
            dma_x4(nc.scalar, 1, 3)
            nc.sync.dma_start(wo_sb[:], wo.rearrange("p (c n) -> p c n", c=2))

            # ---- persistent activations ----
            # qt[:, w, :]: heads (2w, 2w+1) stacked on partitions
            qt = persist.tile([P, 2, S], BF16, tag="qt")
            # K^T zero-padded to full 128-row contraction: [KT;0] and [0;KT]
            kt_lo = persist.tile([P, S], BF16, tag="ktlo")
            kt_hi = persist.tile([P, S], BF16, tag="kthi")
            nc.gpsimd.memset(kt_lo[HD:P, :], 0.0)
            nc.gpsimd.memset(kt_hi[0:HD, :], 0.0)
            # v_aug[:, jc, :]: [V_block (64) | ones (64)]
            v_aug = persist.tile([P, JC, P], BF16, tag="vaug")
            nc.gpsimd.memset(v_aug[:, :, HD:P], 1.0)

            # ================= filler machinery =================
            fill_q = deque()

            def fill(target_ns):
                got = 0.0
                while fill_q and got < target_ns:
                    cost, emit = fill_q.popleft()
                    emit()
                    got += cost

            def drain():
                while fill_q:
                    _, emit = fill_q.popleft()
                    emit()

            def act_copy(dst, src):
                nc.scalar.activation(dst, src, AF.Copy)

            def dve_copy(dst, src):
                nc.vector.tensor_copy(dst, src)

            # ================= projections =================
            def proj_kv(nt, copy_eng, rot_tag):
                """K/V proj + K-RoPE units for cols [512nt, ...).
                Returns (mms_lo, mms_hi, trail): 4+4 matmul units and the
                [copy, rot, rope] trailing units."""
                sl = slice(nt * 512, (nt + 1) * 512)
                st = {}

                def mk(k):
                    def f():
                        if k == 0:
                            st["ps"] = ps.tile(
                                [P, 512], F32, tag="bg", bufs=2, name="kv_ps"
                            )
                        nc.tensor.matmul(
                            st["ps"][:],
                            wkv_sb[:, k, :],
                            xT4_sb[k // 4][:, k % 4, sl],
                            start=(k == 0),
                            stop=(k == KC - 1),
                        )

                    return f

                def cp():
                    kv_raw = sb_tmp.tile(
                        [P, 512], BF16, tag="kvraw", bufs=2, name="kv_raw"
                    )
                    st["raw"] = kv_raw
                    copy_eng(kv_raw[:], st["ps"][:])

                def rot():
                    if rot_tag == "av":  # prologue: borrow the idle av bank
                        rt = ps.tile([P, 2, 512], F32, tag="av", bufs=1, name="rk_ps")
                        rk = rt[0:HD, 0, 0:512]
                    else:
                        rt = ps.tile([P, 512], F32, tag="bg", bufs=2, name="rk_ps")
                        rk = rt[0:HD, 0:512]
                    st["rot"] = rk
                    nc.tensor.matmul(
                        rk, r2t_sb[0:HD, 0:HD], st["raw"][0:HD, :],
                        start=True, stop=True,
                    )
                    # V natural: transpose bf16 V^T blocks straight into v_aug
                    for j in range(4):
                        nc.sync.dma_start_transpose(
                            v_aug[:, nt * 4 + j, 0:HD],
                            st["raw"][HD:P, j * P : (j + 1) * P],
                        )

                def rope():
                    t1k = sb_tmp.tile([HD, 512], BF16, tag="t1", bufs=2, name="t1k")
                    nc.vector.tensor_mul(t1k[:], st["raw"][0:HD, :], cos_sb[0:HD, sl])
                    t2k = sb_tmp.tile([HD, 512], BF16, tag="t2", bufs=2, name="t2k")
                    nc.vector.tensor_mul(t2k[:], st["rot"], sin_sb[0:HD, sl])
                    nc.vector.tensor_add(kt_lo[0:HD, sl], t1k[:], t2k[:])
                    nc.gpsimd.tensor_copy(kt_hi[HD:P, sl], kt_lo[0:HD, sl])

                mms = [(213, mk(k)) for k in range(KC)]
                return mms[:4], mms[4:], [(0, cp), (213, rot), (0, rope)]

            def proj_q(nt, w, copy_eng, rot_tag):
                """Q proj + RoPE units for head pair w, cols [512nt, ...)."""
                sl = slice(nt * 512, (nt + 1) * 512)
                st = {}

                def mk(k):
                    def f():
                        if k == 0:
                            st["ps"] = ps.tile(
                                [P, 512], F32, tag="bg", bufs=2, name="q_ps"
                            )
                        nc.tensor.matmul(
                            st["ps"][:],
                            wq_sb[:, k, w * P : (w + 1) * P],
                            xT4_sb[k // 4][:, k % 4, sl],
                            start=(k == 0),
                            stop=(k == KC - 1),
                        )

                    return f

                def cp():
                    q_raw = sb_tmp.tile(
                        [P, 512], BF16, tag="qraw", bufs=2, name="q_raw"
                    )
                    st["raw"] = q_raw
                    copy_eng(q_raw[:], st["ps"][:])

                def rot():
                    if rot_tag == "av":  # prologue: borrow the idle av bank
                        rt = ps.tile([P, 2, 512], F32, tag="av", bufs=1, name="rq_ps")
                        rp = rt[:, 0, 0:512]
                    else:
                        rt = ps.tile([P, 512], F32, tag="bg", bufs=2, name="rq_ps")
                        rp = rt[:, 0:512]
                    st["rot"] = rp
                    nc.tensor.matmul(
                        rp, r2t_sb[:], st["raw"][:], start=True, stop=True
                    )

                def rope():
                    t1 = sb_tmp.tile([P, 512], BF16, tag="t1", bufs=2, name="t1q")
                    nc.vector.tensor_mul(t1[:], st["raw"][:], cos_sb[:, sl])
                    t2 = sb_tmp.tile([P, 512], BF16, tag="t2", bufs=2, name="t2q")
                    nc.vector.tensor_mul(t2[:], st["rot"], sin_sb[:, sl])
                    nc.vector.tensor_add(qt[:, w, sl], t1[:], t2[:])

                mms = [(213, mk(k)) for k in range(KC)]
                return mms[:4], mms[4:], [(0, cp), (213, rot), (0, rope)]

            def proj_inline0():
                """proj(0) emitted inline, halves interleaved to track the
                per-chunk DMA arrival (u0 -> wq -> u4), trails staggered."""
                kv_lo, kv_hi, kv_tr = proj_kv(0, act_copy, "av")
                q0_lo, q0_hi, q0_tr = proj_q(0, 0, act_copy, "av")
                q1_lo, q1_hi, q1_tr = proj_q(0, 1, act_copy, "av")
                seq = (
                    kv_lo + q0_lo + q1_lo + kv_hi + q0_hi + kv_tr + q1_hi
                    + q0_tr + q1_tr
                )
                for _, f in seq:
                    f()

            def proj_enqueue(nt):
                # filler projs: copies on DVE (Act is busy with Exp); trails
                # staggered a full 8-mm group behind their copy
                kv_lo, kv_hi, kv_tr = proj_kv(nt, dve_copy, "bg")
                q0_lo, q0_hi, q0_tr = proj_q(nt, 0, dve_copy, "bg")
                q1_lo, q1_hi, q1_tr = proj_q(nt, 1, dve_copy, "bg")
                fill_q.extend(
                    kv_lo + kv_hi + q0_lo + q0_hi + kv_tr + q1_lo + q1_hi
                    + q0_tr + q1_tr
                )

            # ================= output projection =================
            ot_store = {}

            def wo_units(nt):
                """Wo for tile nt -> filler units (2 matmuls + copy + dma)."""
                i0 = nt * 512
                ot = ot_store[nt]
                units = []
                for ic in range(4):
                    csl = slice(ic * P, (ic + 1) * P)
                    r0 = i0 + ic * P
                    st = {}

                    def mk_mm(csl, ntile, st):
                        def f():
                            if ntile == 0:
                                st["osb"] = sb_out.tile(
                                    [P, D], BF16, tag="osb", name="out_sb"
                                )
                            wo_ps = ps.tile(
                                [P, 512], F32, tag="bg", bufs=2, name="wo_ps"
                            )
                            st["ps"] = wo_ps
                            nsl = slice(ntile * 512, (ntile + 1) * 512)
                            nc.tensor.matmul(
                                wo_ps[:], ot[0][:, csl], wo_sb[:, 0, nsl],
                                start=True, stop=False,
                            )
                            nc.tensor.matmul(
                                wo_ps[:], ot[1][:, csl], wo_sb[:, 1, nsl],
                                start=False, stop=True,
                            )

                        return f

                    def mk_cp(r0, ntile, st):
                        def f():
                            nsl = slice(ntile * 512, (ntile + 1) * 512)
                            nc.vector.tensor_copy(st["osb"][:, nsl], st["ps"][:])
                            if ntile == 1:
                                nc.sync.dma_start(out[r0 : r0 + P, :], st["osb"][:])

                        return f

                    for ntile in range(2):
                        units.append((426, mk_mm(csl, ntile, st)))
                        units.append((0, mk_cp(r0, ntile, st)))
                return units

            # wo(3) split around attn(3)'s h-boundary
            wo3_st = {}

            def wo3_mid(ot0):
                """h0 half of wo(3): runs as attn(3) mid-tile filler."""
                acc = sb_ot.tile([P, 4, D], BF16, tag="woacc", bufs=1, name="woacc")
                wo3_st["acc"] = acc
                for ic in range(4):
                    csl = slice(ic * P, (ic + 1) * P)
                    for ntile in range(2):
                        nsl = slice(ntile * 512, (ntile + 1) * 512)
                        wo_ps = ps.tile([P, 512], F32, tag="bg", bufs=2, name="wo_ps")
                        nc.tensor.matmul(
                            wo_ps[:], ot0[:, csl], wo_sb[:, 0, nsl],
                            start=True, stop=True,
                        )
                        nc.vector.tensor_copy(acc[:, ic, nsl], wo_ps[:])

            def wo3_tail():
                """h1 half of wo(3) + combine: the only post-attn work."""
                ot = ot_store[3]
                acc = wo3_st["acc"]
                for ic in range(4):
                    csl = slice(ic * P, (ic + 1) * P)
                    r0 = 1536 + ic * P
                    osb = sb_out.tile([P, D], BF16, tag="osb", name="out_sb")
                    for ntile in range(2):
                        nsl = slice(ntile * 512, (ntile + 1) * 512)
                        wo_ps = ps.tile([P, 512], F32, tag="bg", bufs=2, name="wo_ps")
                        nc.tensor.matmul(
                            wo_ps[:], ot[1][:, csl], wo_sb[:, 1, nsl],
                            start=True, stop=True,
                        )
                        nc.vector.tensor_add(
                            osb[:, nsl], wo_ps[:], acc[:, ic, nsl]
                        )
                    eng = nc.scalar if ic == 3 else nc.sync
                    eng.dma_start(out[r0 : r0 + P, :], osb[:])

            # ================= attention =================
            def attn_tile(i0, iw, mid_fn=None):
                """attention for queries [i0, i0+iw); Wo deferred via units."""
                nfull = i0 // P
                # diagonal pieces first: r=0 initializes the full av region
                # (start=True), r>=1 accumulate into sub-regions; trailing
                # full pieces then keep the Exp->AV chain mask-free
                pieces = [(nfull + r, P * r) for r in range(iw // P)] + [
                    (jc, 0) for jc in range(nfull)
                ]
                ot = []
                for h, kt in ((0, kt_lo), (1, kt_hi)):
                    # av: heads (h, 2+h) in w slices; rows 64:128 = denom
                    av = ps.tile([P, 2, iw], F32, tag="av", bufs=1, name="av")
                    for pi, (jc, ls) in enumerate(pieces):
                        n = iw - ls
                        isl = slice(i0 + ls, i0 + iw)
                        jsl = slice(jc * P, (jc + 1) * P)
                        diag = jc >= nfull
                        sc = ps.tile([P, 2, iw], F32, tag="sc", bufs=2, name="sc")
                        for w in range(2):
                            nc.tensor.matmul(
                                sc[:, w, 0:n],
                                kt[:, jsl],
                                qt[:, w, isl],
                                start=True,
                                stop=not diag,
                            )
                        if diag:  # diagonal chunk: PE-accumulate -1e9 mask
                            nc.tensor.matmul(
                                sc[:, :, 0:P],
                                trineg_sb[:],
                                i2_sb[:],
                                start=False,
                                stop=True,
                                skip_group_check=True,
                            )
                        pt = sb_pt.tile([P, 2, 512], BF16, tag="pt", name="pt")
                        nc.scalar.activation(
                            pt[:, :, 0:n], sc[:, :, 0:n], AF.Exp, scale=0.125
                        )
                        for w in range(2):
                            nc.tensor.matmul(
                                av[:, w, ls:iw],
                                v_aug[:, jc, :],
                                pt[:, w, 0:n],
                                start=(pi == 0),
                                stop=(pi == len(pieces) - 1),
                            )
                        # keep the PE fed: Act's Exp runs ~185ns/piece longer
                        # than the piece's PE work
                        fill(213)

                    # normalize -> ot chunk h (rows: [head h | head 2+h])
                    rec = sb_tmp.tile([P, 2, 512], F32, tag="rec", bufs=2, name="rec")
                    ot_h = sb_ot.tile([P, 512], BF16, tag=f"ot{h}", bufs=2, name="ot_h")
                    for w in range(2):
                        nc.vector.reciprocal_approx_fast(
                            rec[:, w, 0:iw], av[:, w, :]
                        )
                        nc.vector.tensor_mul(
                            ot_h[w * HD : (w + 1) * HD, 0:iw],
                            av[0:HD, w, :],
                            rec[HD:P, w, 0:iw],
                        )
                    ot.append(ot_h)
                    # cover the av-bank hold (DVE rec+mul) before the next
                    # h-pass / tile can start accumulating
                    fill(2400)
                    if h == 0 and mid_fn is not None:
                        mid_fn(ot[0])
                ot_store[i0 // 512] = ot

            # ================= schedule =================
            proj_inline0()
            proj_enqueue(1)
            attn_tile(0, 512)
            drain()  # proj(1) leftovers
            proj_enqueue(2)
            attn_tile(512, 512)
            drain()  # proj(2) leftovers
            proj_enqueue(3)
            fill_q.extend(wo_units(0))
            attn_tile(1024, 512)
            drain()
            u2 = wo_units(2)
            fill_q.extend(wo_units(1))
            fill_q.extend(u2[:8])
            attn_tile(1536, 512, mid_fn=wo3_mid)
            drain()
            # reserved wo(2) second half covers the final h1 rec/mul hold
            for _, f in u2[8:]:
                f()
            wo3_tail()

    nc.compile()
    return nc


def _prep_inputs(x, cos, sin, Wq, Wk, Wv, Wo):
    """Build per-core input maps (host-side sharding + layout), all bf16.

    All weight/table tensors are pre-packed partition-major so each DMA line
    per partition is one contiguous 2-4KB run (fast descriptor generation).
    """
    import ml_dtypes

    bf16 = ml_dtypes.bfloat16

    cos2 = np.ascontiguousarray(cos.T).astype(bf16)  # (64, 2048)
    sin2 = np.ascontiguousarray(sin.T).astype(bf16)

    # lhsT of rotate_half: rot(q) = R q, r2t = R^T (block-diag over 2 heads)
    r2t = np.zeros((P, P), dtype=np.float32)
    for o in (0, HD):
        for e in range(32):
            r2t[o + e, o + e + 32] = 1.0
        for e in range(32, HD):
            r2t[o + e, o + e - 32] = -1.0

    # causal mask accumulated on the PE: trineg[k,m] = -1e9 where m>k;
    # i2 = identity duplicated for the (w,i) paired rhs
    trineg = np.where(
        np.arange(P)[None, :] > np.arange(P)[:, None], -1e9, 0.0
    ).astype(bf16)
    eye = np.eye(P, dtype=np.float32)
    i2 = np.ascontiguousarray(
        np.stack([eye, eye], axis=1).reshape(P, 2 * P)
    ).astype(bf16)

    # x4[g*4+b][p][(j c)] = x^T[(4g+j)*128+p, 512b+c]: 4 chunks fused per
    # transfer so each partition's DMA line is 4KB contiguous
    x4 = []
    for b_ in range(B):
        xt = np.ascontiguousarray(x[b_].T).astype(bf16)  # [D, S]
        x4.append(
            np.ascontiguousarray(
                xt.reshape(2, 4, P, 4, 512)
                .transpose(0, 3, 2, 1, 4)
                .reshape(8, P, 4 * 512)
            )
        )

    in_maps = []
    for c in range(N_CORES):
        b, g = c // KV, c % KV
        # wo rows reordered to match ot chunk layout: c0=[h0,h2], c1=[h1,h3]
        wo_g = Wo[g * NH * HD : (g + 1) * NH * HD, :].reshape(NH, HD, D)
        wo_perm = np.concatenate([wo_g[0], wo_g[2], wo_g[1], wo_g[3]], axis=0)
        # partition-major packs: [K, M] -> [P, KC, M] -> [P, KC*M]
        wq_g = Wq[:, g * NH * HD : (g + 1) * NH * HD]
        wq_pack = (
            wq_g.reshape(KC, P, NH * HD).transpose(1, 0, 2).reshape(P, -1)
        )
        wkv_g = np.concatenate(
            [Wk[:, g * HD : (g + 1) * HD], Wv[:, g * HD : (g + 1) * HD]], axis=1
        )
        wkv_pack = (
            wkv_g.reshape(KC, P, 2 * HD).transpose(1, 0, 2).reshape(P, -1)
        )
        wo_pack = wo_perm.reshape(2, P, D).transpose(1, 0, 2).reshape(P, -1)
        in_maps.append(
            {
                "x4": x4[b],
                "wq": np.ascontiguousarray(wq_pack).astype(bf16),
                "wkv": np.ascontiguousarray(wkv_pack).astype(bf16),
                "wo": np.ascontiguousarray(wo_pack).astype(bf16),
                "cos2": cos2,
                "sin2": sin2,
                "r2t": r2t.astype(bf16),
                "trineg": trineg,
                "i2": i2,
            }
        )
    return in_maps


def kernel(x, cos, sin, Wq, Wk, Wv, Wo):
    x = np.asarray(x, dtype=np.float32)
    cos = np.asarray(cos, dtype=np.float32)
    sin = np.asarray(sin, dtype=np.float32)
    Wq = np.asarray(Wq, dtype=np.float32)
    Wk = np.asarray(Wk, dtype=np.float32)
    Wv = np.asarray(Wv, dtype=np.float32)
    Wo = np.asarray(Wo, dtype=np.float32)

    trace = os.environ.get("TRN_TRACE", "") == "1"
    if trace:
        _install_trace_hook()

    if "nc" not in _cached:
        _cached["nc"] = build_bass()
    nc = _cached["nc"]

    in_maps = _prep_inputs(x, cos, sin, Wq, Wk, Wv, Wo)
    res = run_bass_kernel_spmd(nc, in_maps, list(range(N_CORES)), trace=trace)
    if trace and res.exec_time_ns is not None:
        print(f"HW exec time: {res.exec_time_ns} ns")
        _cached["exec_time_ns"] = res.exec_time_ns
        _cached["trace_path"] = (
            res.instructions_and_trace[1] if res.instructions_and_trace else None
        )

    out = np.zeros((B, S, D), dtype=np.float32)
    for c in range(N_CORES):
        out[c // KV] += np.asarray(res.results[c]["out"], dtype=np.float32)
    return out


# revision 54
# speedup vs baseline: 1.0728x; 1.0728x over previous
"""Causal GQA attention block (RoPE, 16 q-heads / 4 kv-heads, D=1024, S=2048, B=2)
distributed over 8 NeuronCores: data-parallel over batch (2) x tensor-parallel
over kv-groups (4). Each core computes 4 query heads + 1 kv head for one batch
element, Megatron-style: Wq/Wk/Wv column-parallel, Wo row-parallel with the
row-parallel partial sums reduced on host.

v6 scheduling notes (on top of the v2 layout):
 - The input stream (~5.4MB) runs at ~210GB/s real => ~25us DMA wall; the
   whole schedule hugs it: DMA strictly by first need, x packed 4-chunks-
   per-transfer (4KB lines), cos/sin tables halved (de-duplicated), PE
   warmup matmuls ramp the DVFS p-state while the first megabyte streams.
 - Attention starts right after proj(0); proj(1) becomes filler inside
   attn(0). Every proj/Wo is chopped into ~213-426ns units trickled one per
   attention piece (the Exp on Act runs ~185ns/piece longer than the
   piece's PE work) and in ~2.4us lumps at the softmax-normalize (DVE
   rec+mul) boundaries where the av PSUM bank is held.
 - All proj groups emit stop->copy->rotate staggered one 4-matmul half
   behind, so the PE never waits on an Act/DVE copy; prologue rot psums
   borrow the idle av bank.
 - Causal mask is accumulated on the PE itself (trineg lhsT x identity-pair
   rhs into the score PSUM) - no cross-engine mask hop.
 - wo(3) splits around attn(3)'s h-boundary: the h0 half runs as mid-tile
   filler, the h1 half is the only tail work after the last reciprocal.
"""

import os
import sys
import types
from collections import deque

import numpy as np

import concourse.bass as bass
import concourse.mybir as mybir
import concourse.tile as tile
from concourse import bacc
from concourse.bass_utils import run_bass_kernel_spmd

F32 = mybir.dt.float32
BF16 = mybir.dt.bfloat16
AF = mybir.ActivationFunctionType

B, S, D = 2, 2048, 1024
H, KV, HD = 16, 4, 64
NH = 4  # query heads per core
P = 128
NT = S // 512  # 4 i-tiles of 512
KC = D // P  # 8 contraction chunks
JC = S // P  # 16 j-chunks
N_CORES = 8
N_WARM = 12  # PE warmup matmuls (DVFS ramp) while the DMA prologue streams

_cached = {}


def _install_trace_hook():
    """NTFF profiling hook shim (the container's antenv lacks axon_hooks)."""
    try:
        import antenv

        if "antenv.axon_hooks" in sys.modules:
            return
        mod = types.ModuleType("antenv.axon_hooks")
        _h = [None]
        mod.set_axon_ntff_profile_hook = lambda h: _h.__setitem__(0, h)
        mod.get_axon_ntff_profile_hook = lambda: _h[0]
        sys.modules["antenv.axon_hooks"] = mod
        antenv.axon_hooks = mod
        from trn_agent_boot.trn_boot import _ntff_profile_via_ctypes

        mod.set_axon_ntff_profile_hook(
            _ntff_profile_via_ctypes("/opt/axon/libaxon_pjrt.so")
        )
    except Exception:
        pass


def build_bass():
    nc = bacc.Bacc("TRN2", target_bir_lowering=False, debug=False, num_devices=N_CORES)

    # x4[g*4+b] = fused cols [512b, 512b+512) of chunks 4g..4g+3: 4KB DMA
    # lines per partition (1KB lines ran at ~half the DMA roofline)
    x4 = nc.dram_tensor("x4", [8, P, 4 * 512], BF16, kind="ExternalInput")
    # host pre-packed partition-major: one contiguous line per partition
    wq = nc.dram_tensor("wq", [P, KC * NH * HD], BF16, kind="ExternalInput")
    wkv = nc.dram_tensor("wkv", [P, KC * 2 * HD], BF16, kind="ExternalInput")
    wo = nc.dram_tensor("wo", [P, 2 * D], BF16, kind="ExternalInput")
    cos2 = nc.dram_tensor("cos2", [P, S], BF16, kind="ExternalInput")
    sin2 = nc.dram_tensor("sin2", [P, S], BF16, kind="ExternalInput")
    r2t = nc.dram_tensor("r2t", [P, P], BF16, kind="ExternalInput")
    # causal mask as PE accumulate: sc[j,(w,i)] += trineg[i,j] via identity
    # rhs pair; trineg[k,m] = -1e9 where m>k else 0
    trineg = nc.dram_tensor("trineg", [P, P], BF16, kind="ExternalInput")
    i2 = nc.dram_tensor("i2", [P, 2 * P], BF16, kind="ExternalInput")
    out = nc.dram_tensor("out", [S, D], BF16, kind="ExternalOutput")

    with tile.TileContext(nc) as tc:
        with (
            tc.tile_pool(name="const", bufs=1) as const,
            tc.tile_pool(name="persist", bufs=1) as persist,
            tc.tile_pool(name="sb_tmp", bufs=4) as sb_tmp,
            tc.tile_pool(name="sb_pt", bufs=8) as sb_pt,
            tc.tile_pool(name="sb_ot", bufs=4) as sb_ot,
            tc.tile_pool(name="sb_out", bufs=4) as sb_out,
            tc.tile_pool(name="ps", bufs=1, space="PSUM") as ps,
        ):
            # ---- PE warmup: memset tile -> matmul stream (ramps p-state) ----
            warm_sb = const.tile([P, 512], BF16)
            nc.gpsimd.memset(warm_sb[:], 0.0)
            for _ in range(N_WARM):
                wps = ps.tile([P, 512], F32, tag="bg", bufs=2, name="warm_ps")
                nc.tensor.matmul(
                    wps[:], warm_sb[:, 0:P], warm_sb[:], start=True, stop=True
                )

            # ---- DMA prologue: strict first-need order on 2 queues ----
            # xT lives as 2 fused tiles: chunk k = xT4_sb[k//4][:, k%4, :]
            xT4_sb = [
                persist.tile([P, 4, S], BF16, tag=f"xT4_{g}", name=f"xT4_sb{g}")
                for g in range(2)
            ]
            wkv_sb = const.tile([P, KC, 2 * HD], BF16)
            cos_sb = const.tile([P, S], BF16)
            sin_sb = const.tile([P, S], BF16)
            r2t_sb = const.tile([P, P], BF16)
            wq_sb = const.tile([P, KC, NH * HD], BF16)
            trineg_sb = const.tile([P, P], BF16)
            i2_sb = const.tile([P, 2, P], BF16)
            wo_sb = const.tile([P, 2, D], BF16)

            def dma_x4(eng, g, b):
                eng.dma_start(
                    xT4_sb[g][:, :, b * 512 : (b + 1) * 512],
                    x4[g * 4 + b].rearrange("p (j c) -> p j c", j=4),
                )

            nc.sync.dma_start(wkv_sb[:], wkv.rearrange("p (k m) -> p k m", k=KC))
            dma_x4(nc.scalar, 0, 0)
            nc.scalar.dma_start(wq_sb[:], wq.rearrange("p (k m) -> p k m", k=KC))
            dma_x4(nc.sync, 1, 0)
            nc.sync.dma_start(r2t_sb[:], r2t[:])
            nc.scalar.dma_start(trineg_sb[:], trineg[:])
            nc.scalar.dma_start(i2_sb[:], i2.rearrange("p (c m) -> p c m", c=2))
            nc.sync.dma_start(cos_sb[:], cos2[:])
            nc.scalar.dma_start(sin_sb[:], sin2[:])
            dma_x4(nc.scalar, 0, 1)
            dma_x4(nc.sync, 1, 1)
            dma_x4(nc.sync, 0, 2)
            dma_x4(nc.scalar, 1, 2)
            dma_x4(nc.sync, 0, 3)
            dma_x4(nc.scalar, 1, 3)
            nc.sync.dma_start(wo_sb[:], wo.rearrange("p (c n) -> p c n", c=2))

            # ---- persistent activations ----
            # qt[:, w, :]: heads (2w, 2w+1) stacked on partitions
            qt = persist.tile([P, 2, S], BF16, tag="qt")
            # K^T zero-padded to full 128-row contraction: [KT;0] and [0;KT]
            kt_lo = persist.tile([P, S], BF16, tag="ktlo")
            kt_hi = persist.tile([P, S], BF16, tag="kthi")
            nc.gpsimd.memset(kt_lo[HD:P, :], 0.0)
            nc.gpsimd.memset(kt_hi[0:HD, :], 0.0)
            # v_aug[:, jc, :]: [V_block (64) | ones (64)]
            v_aug = persist.tile([P, JC, P], BF16, tag="vaug")
            nc.gpsimd.memset(v_aug[:, :, HD:P], 1.0)

            # ================= filler machinery =================
            fill_q = deque()

            def fill(target_ns):
                got = 0.0
                while fill_q and got < target_ns:
                    cost, emit = fill_q.popleft()
                    emit()
                    got += cost

            def drain():
                while fill_q:
                    _, emit = fill_q.popleft()
                    emit()

            def act_copy(dst, src):
                nc.scalar.activation(dst, src, AF.Copy)

            def dve_copy(dst, src):
                nc.vector.tensor_copy(dst, src)

            # ================= projections =================
            def proj_kv(nt, copy_eng, rot_tag):
                """K/V proj + K-RoPE units for cols [512nt, ...).
                Returns (mms_lo, mms_hi, trail): 4+4 matmul units and the
                [copy, rot, rope] trailing units."""
                sl = slice(nt * 512, (nt + 1) * 512)
                st = {}

                def mk(k):
                    def f():
                        if k == 0:
                            st["ps"] = ps.tile(
                                [P, 512], F32, tag="bg", bufs=2, name="kv_ps"
                            )
                        nc.tensor.matmul(
                            st["ps"][:],
                            wkv_sb[:, k, :],
                            xT4_sb[k // 4][:, k % 4, sl],
                            start=(k == 0),
                            stop=(k == KC - 1),
                        )

                    return f

                def cp():
                    kv_raw = sb_tmp.tile(
                        [P, 512], BF16, tag="kvraw", bufs=2, name="kv_raw"
                    )
                    st["raw"] = kv_raw
                    copy_eng(kv_raw[:], st["ps"][:])

                def rot():
                    if rot_tag == "av":  # prologue: borrow the idle av bank
                        rt = ps.tile([P, 2, 512], F32, tag="av", bufs=1, name="rk_ps")
                        rk = rt[0:HD, 0, 0:512]
                    else:
                        rt = ps.tile([P, 512], F32, tag="bg", bufs=2, name="rk_ps")
                        rk = rt[0:HD, 0:512]
                    st["rot"] = rk
                    nc.tensor.matmul(
                        rk, r2t_sb[0:HD, 0:HD], st["raw"][0:HD, :],
                        start=True, stop=True,
                    )
                    # V natural: transpose bf16 V^T blocks straight into v_aug
                    for j in range(4):
                        nc.sync.dma_start_transpose(
                            v_aug[:, nt * 4 + j, 0:HD],
                            st["raw"][HD:P, j * P : (j + 1) * P],
                        )

                def rope():
                    t1k = sb_tmp.tile([HD, 512], BF16, tag="t1", bufs=2, name="t1k")
                    nc.vector.tensor_mul(t1k[:], st["raw"][0:HD, :], cos_sb[0:HD, sl])
                    t2k = sb_tmp.tile([HD, 512], BF16, tag="t2", bufs=2, name="t2k")
                    nc.vector.tensor_mul(t2k[:], st["rot"], sin_sb[0:HD, sl])
                    nc.vector.tensor_add(kt_lo[0:HD, sl], t1k[:], t2k[:])
                    nc.gpsimd.tensor_copy(kt_hi[HD:P, sl], kt_lo[0:HD, sl])

                mms = [(213, mk(k)) for k in range(KC)]
                return mms[:4], mms[4:], [(0, cp), (213, rot), (0, rope)]

            def proj_q(nt, w, copy_eng, rot_tag):
                """Q proj + RoPE units for head pair w, cols [512nt, ...)."""
                sl = slice(nt * 512, (nt + 1) * 512)
                st = {}

                def mk(k):
                    def f():
                        if k == 0:
                            st["ps"] = ps.tile(
                                [P, 512], F32, tag="bg", bufs=2, name="q_ps"
                            )
                        nc.tensor.matmul(
                            st["ps"][:],
                            wq_sb[:, k, w * P : (w + 1) * P],
                            xT4_sb[k // 4][:, k % 4, sl],
                            start=(k == 0),
                            stop=(k == KC - 1),
                        )

                    return f

                def cp():
                    q_raw = sb_tmp.tile(
                        [P, 512], BF16, tag="qraw", bufs=2, name="q_raw"
                    )
                    st["raw"] = q_raw
                    copy_eng(q_raw[:], st["ps"][:])

                def rot():
                    if rot_tag == "av":  # prologue: borrow the idle av bank
                        rt = ps.tile([P, 2, 512], F32, tag="av", bufs=1, name="rq_ps")
                        rp = rt[:, 0, 0:512]
                    else:
                        rt = ps.tile([P, 512], F32, tag="bg", bufs=2, name="rq_ps")
                        rp = rt[:, 0:512]
                    st["rot"] = rp
                    nc.tensor.matmul(
                        rp, r2t_sb[:], st["raw"][:], start=True, stop=True
                    )

                def rope():
                    t1 = sb_tmp.tile([P, 512], BF16, tag="t1", bufs=2, name="t1q")
                    nc.vector.tensor_mul(t1[:], st["raw"][:], cos_sb[:, sl])
                    t2 = sb_tmp.tile([P, 512], BF16, tag="t2", bufs=2, name="t2q")
                    nc.vector.tensor_mul(t2[:], st["rot"], sin_sb[:, sl])
                    nc.vector.tensor_add(qt[:, w, sl], t1[:], t2[:])

                mms = [(213, mk(k)) for k in range(KC)]
                return mms[:4], mms[4:], [(0, cp), (213, rot), (0, rope)]

            def proj_inline_pair(ntA, ntB):
                """proj(ntA)+proj(ntB) inline, interleaved at group level:
                trails (copy/rot/rope) run one 8-matmul group late so the PE
                never waits on an Act copy, and PSUM slot reuse is always a
                full group (~1.7us) stale. The first two groups interleave
                their 4-matmul halves to track per-chunk DMA arrival."""
                kvA = proj_kv(ntA, act_copy, "av")
                kvB = proj_kv(ntB, act_copy, "av")
                qA0 = proj_q(ntA, 0, act_copy, "av")
                qA1 = proj_q(ntA, 1, act_copy, "av")
                qB0 = proj_q(ntB, 0, act_copy, "av")
                qB1 = proj_q(ntB, 1, act_copy, "av")
                seq = (
                    kvA[0] + qA0[0] + kvA[1] + qA0[1] + kvA[2]
                    + qA1[0] + qA1[1] + qA0[2]
                    + kvB[0] + kvB[1] + qA1[2]
                    + qB0[0] + qB0[1] + kvB[2]
                    + qB1[0] + qB1[1] + qB0[2] + qB1[2]
                )
                for _, f in seq:
                    f()

            def proj_enqueue(nt):
                # filler projs: copies on DVE (Act is busy with Exp); trails
                # staggered a full 8-mm group behind their copy
                kv_lo, kv_hi, kv_tr = proj_kv(nt, dve_copy, "bg")
                q0_lo, q0_hi, q0_tr = proj_q(nt, 0, dve_copy, "bg")
                q1_lo, q1_hi, q1_tr = proj_q(nt, 1, dve_copy, "bg")
                fill_q.extend(
                    kv_lo + kv_hi + q0_lo + q0_hi + kv_tr + q1_lo + q1_hi
                    + q0_tr + q1_tr
                )

            # ================= output projection =================
            ot_store = {}

            def wo_units(nt):
                """Wo for tile nt -> filler units (2 matmuls + copy + dma)."""
                i0 = nt * 512
                ot = ot_store[nt]
                units = []
                for ic in range(4):
                    csl = slice(ic * P, (ic + 1) * P)
                    r0 = i0 + ic * P
                    st = {}

                    def mk_mm(csl, ntile, st):
                        def f():
                            if ntile == 0:
                                st["osb"] = sb_out.tile(
                                    [P, D], BF16, tag="osb", name="out_sb"
                                )
                            wo_ps = ps.tile(
                                [P, 512], F32, tag="bg", bufs=2, name="wo_ps"
                            )
                            st["ps"] = wo_ps
                            nsl = slice(ntile * 512, (ntile + 1) * 512)
                            nc.tensor.matmul(
                                wo_ps[:], ot[0][:, csl], wo_sb[:, 0, nsl],
                                start=True, stop=False,
                            )
                            nc.tensor.matmul(
                                wo_ps[:], ot[1][:, csl], wo_sb[:, 1, nsl],
                                start=False, stop=True,
                            )

                        return f

                    def mk_cp(r0, ntile, st):
                        def f():
                            nsl = slice(ntile * 512, (ntile + 1) * 512)
                            nc.vector.tensor_copy(st["osb"][:, nsl], st["ps"][:])
                            if ntile == 1:
                                nc.sync.dma_start(out[r0 : r0 + P, :], st["osb"][:])

                        return f

                    for ntile in range(2):
                        units.append((426, mk_mm(csl, ntile, st)))
                        units.append((0, mk_cp(r0, ntile, st)))
                return units

            # wo(3) split around attn(3)'s h-boundary
            wo3_st = {}

            def wo3_mid(ot0):
                """h0 half of wo(3): runs as attn(3) mid-tile filler."""
                acc = sb_ot.tile([P, 4, D], BF16, tag="woacc", bufs=1, name="woacc")
                wo3_st["acc"] = acc
                for ic in range(4):
                    csl = slice(ic * P, (ic + 1) * P)
                    for ntile in range(2):
                        nsl = slice(ntile * 512, (ntile + 1) * 512)
                        wo_ps = ps.tile([P, 512], F32, tag="bg", bufs=2, name="wo_ps")
                        nc.tensor.matmul(
                            wo_ps[:], ot0[:, csl], wo_sb[:, 0, nsl],
                            start=True, stop=True,
                        )
                        nc.vector.tensor_copy(acc[:, ic, nsl], wo_ps[:])

            def wo3_tail():
                """h1 half of wo(3) + combine: the only post-attn work."""
                ot = ot_store[3]
                acc = wo3_st["acc"]
                for ic in range(4):
                    csl = slice(ic * P, (ic + 1) * P)
                    r0 = 1536 + ic * P
                    osb = sb_out.tile([P, D], BF16, tag="osb", name="out_sb")
                    for ntile in range(2):
                        nsl = slice(ntile * 512, (ntile + 1) * 512)
                        wo_ps = ps.tile([P, 512], F32, tag="bg", bufs=2, name="wo_ps")
                        nc.tensor.matmul(
                            wo_ps[:], ot[1][:, csl], wo_sb[:, 1, nsl],
                            start=True, stop=True,
                        )
                        nc.vector.tensor_add(
                            osb[:, nsl], wo_ps[:], acc[:, ic, nsl]
                        )
                    eng = nc.scalar if ic == 3 else nc.sync
                    eng.dma_start(out[r0 : r0 + P, :], osb[:])

            # ================= attention =================
            def attn_tile(i0, iw, mid_fn=None):
                """attention for queries [i0, i0+iw); Wo deferred via units."""
                nfull = i0 // P
                # diagonal pieces first: r=0 initializes the full av region
                # (start=True), r>=1 accumulate into sub-regions; trailing
                # full pieces then keep the Exp->AV chain mask-free
                pieces = [(nfull + r, P * r) for r in range(iw // P)] + [
                    (jc, 0) for jc in range(nfull)
                ]
                ot = []
                for h, kt in ((0, kt_lo), (1, kt_hi)):
                    # av: heads (h, 2+h) in w slices; rows 64:128 = denom
                    av = ps.tile([P, 2, iw], F32, tag="av", bufs=1, name="av")
                    for pi, (jc, ls) in enumerate(pieces):
                        n = iw - ls
                        isl = slice(i0 + ls, i0 + iw)
                        jsl = slice(jc * P, (jc + 1) * P)
                        diag = jc >= nfull
                        sc = ps.tile([P, 2, iw], F32, tag="sc", bufs=2, name="sc")
                        for w in range(2):
                            nc.tensor.matmul(
                                sc[:, w, 0:n],
                                kt[:, jsl],
                                qt[:, w, isl],
                                start=True,
                                stop=not diag,
                            )
                        if diag:  # diagonal chunk: PE-accumulate -1e9 mask
                            nc.tensor.matmul(
                                sc[:, :, 0:P],
                                trineg_sb[:],
                                i2_sb[:],
                                start=False,
                                stop=True,
                                skip_group_check=True,
                            )
                        pt = sb_pt.tile([P, 2, 512], BF16, tag="pt", name="pt")
                        nc.scalar.activation(
                            pt[:, :, 0:n], sc[:, :, 0:n], AF.Exp, scale=0.125
                        )
                        for w in range(2):
                            nc.tensor.matmul(
                                av[:, w, ls:iw],
                                v_aug[:, jc, :],
                                pt[:, w, 0:n],
                                start=(pi == 0),
                                stop=(pi == len(pieces) - 1),
                            )
                        # keep the PE fed: Act's Exp runs ~185ns/piece longer
                        # than the piece's PE work
                        fill(213)

                    # normalize -> ot chunk h (rows: [head h | head 2+h])
                    rec = sb_tmp.tile([P, 2, 512], F32, tag="rec", bufs=2, name="rec")
                    ot_h = sb_ot.tile([P, 512], BF16, tag=f"ot{h}", bufs=2, name="ot_h")
                    for w in range(2):
                        nc.vector.reciprocal_approx_fast(
                            rec[:, w, 0:iw], av[:, w, :]
                        )
                        nc.vector.tensor_mul(
                            ot_h[w * HD : (w + 1) * HD, 0:iw],
                            av[0:HD, w, :],
                            rec[HD:P, w, 0:iw],
                        )
                    ot.append(ot_h)
                    # cover the av-bank hold (DVE rec+mul) before the next
                    # h-pass / tile can start accumulating
                    fill(2400)
                    if h == 0 and mid_fn is not None:
                        mid_fn(ot[0])
                ot_store[i0 // 512] = ot

            # ================= schedule =================
            proj_inline_pair(0, 1)
            proj_enqueue(2)
            attn_tile(0, 512)
            drain()  # proj(2) leftovers
            proj_enqueue(3)
            attn_tile(512, 512)
            drain()  # proj(3) leftovers
            fill_q.extend(wo_units(0))
            attn_tile(1024, 512)
            drain()
            u2 = wo_units(2)
            fill_q.extend(wo_units(1))
            fill_q.extend(u2[:8])
            attn_tile(1536, 512, mid_fn=wo3_mid)
            drain()
            # reserved wo(2) second half covers the final h1 rec/mul hold
            for _, f in u2[8:]:
                f()
            wo3_tail()

    nc.compile()
    return nc


def _prep_inputs(x, cos, sin, Wq, Wk, Wv, Wo):
    """Build per-core input maps (host-side sharding + layout), all bf16.

    All weight/table tensors are pre-packed partition-major so each DMA line
    per partition is one contiguous 2-4KB run (fast descriptor generation).
    """
    import ml_dtypes

    bf16 = ml_dtypes.bfloat16

    cosT = np.ascontiguousarray(cos.T)  # (64, 2048)
    sinT = np.ascontiguousarray(sin.T)
    cos2 = np.concatenate([cosT, cosT], axis=0).astype(bf16)
    sin2 = np.concatenate([sinT, sinT], axis=0).astype(bf16)

    # lhsT of rotate_half: rot(q) = R q, r2t = R^T (block-diag over 2 heads)
    r2t = np.zeros((P, P), dtype=np.float32)
    for o in (0, HD):
        for e in range(32):
            r2t[o + e, o + e + 32] = 1.0
        for e in range(32, HD):
            r2t[o + e, o + e - 32] = -1.0

    # causal mask accumulated on the PE: trineg[k,m] = -1e9 where m>k;
    # i2 = identity duplicated for the (w,i) paired rhs
    trineg = np.where(
        np.arange(P)[None, :] > np.arange(P)[:, None], -1e9, 0.0
    ).astype(bf16)
    eye = np.eye(P, dtype=np.float32)
    i2 = np.ascontiguousarray(
        np.stack([eye, eye], axis=1).reshape(P, 2 * P)
    ).astype(bf16)

    # x4[g*4+b][p][(j c)] = x^T[(4g+j)*128+p, 512b+c]: 4 chunks fused per
    # transfer so each partition's DMA line is 4KB contiguous
    x4 = []
    for b_ in range(B):
        xt = np.ascontiguousarray(x[b_].T).astype(bf16)  # [D, S]
        x4.append(
            np.ascontiguousarray(
                xt.reshape(2, 4, P, 4, 512)
                .transpose(0, 3, 2, 1, 4)
                .reshape(8, P, 4 * 512)
            )
        )

    in_maps = []
    for c in range(N_CORES):
        b, g = c // KV, c % KV
        # wo rows reordered to match ot chunk layout: c0=[h0,h2], c1=[h1,h3]
        wo_g = Wo[g * NH * HD : (g + 1) * NH * HD, :].reshape(NH, HD, D)
        wo_perm = np.concatenate([wo_g[0], wo_g[2], wo_g[1], wo_g[3]], axis=0)
        # partition-major packs: [K, M] -> [P, KC, M] -> [P, KC*M]
        wq_g = Wq[:, g * NH * HD : (g + 1) * NH * HD]
        wq_pack = (
            wq_g.reshape(KC, P, NH * HD).transpose(1, 0, 2).reshape(P, -1)
        )
        wkv_g = np.concatenate(
            [Wk[:, g * HD : (g + 1) * HD], Wv[:, g * HD : (g + 1) * HD]], axis=1
        )
        wkv_pack = (
            wkv_g.reshape(KC, P, 2 * HD).transpose(1, 0, 2).reshape(P, -1)
        )
        wo_pack = wo_perm.reshape(2, P, D).transpose(1, 0, 2).reshape(P, -1)
        in_maps.append(
            {
                "x4": x4[b],
                "wq": np.ascontiguousarray(wq_pack).astype(bf16),
                "wkv": np.ascontiguousarray(wkv_pack).astype(bf16),
                "wo": np.ascontiguousarray(wo_pack).astype(bf16),
                "cos2": cos2,
                "sin2": sin2,
                "r2t": r2t.astype(bf16),
                "trineg": trineg,
                "i2": i2,
            }
        )
    return in_maps


def kernel(x, cos, sin, Wq, Wk, Wv, Wo):
    x = np.asarray(x, dtype=np.float32)
    cos = np.asarray(cos, dtype=np.float32)
    sin = np.asarray(sin, dtype=np.float32)
    Wq = np.asarray(Wq, dtype=np.float32)
    Wk = np.asarray(Wk, dtype=np.float32)
    Wv = np.asarray(Wv, dtype=np.float32)
    Wo = np.asarray(Wo, dtype=np.float32)

    trace = os.environ.get("TRN_TRACE", "") == "1"
    if trace:
        _install_trace_hook()

    if "nc" not in _cached:
        _cached["nc"] = build_bass()
    nc = _cached["nc"]

    in_maps = _prep_inputs(x, cos, sin, Wq, Wk, Wv, Wo)
    res = run_bass_kernel_spmd(nc, in_maps, list(range(N_CORES)), trace=trace)
    if trace and res.exec_time_ns is not None:
        print(f"HW exec time: {res.exec_time_ns} ns")
        _cached["exec_time_ns"] = res.exec_time_ns
        _cached["trace_path"] = (
            res.instructions_and_trace[1] if res.instructions_and_trace else None
        )

    out = np.zeros((B, S, D), dtype=np.float32)
    for c in range(N_CORES):
        out[c // KV] += np.asarray(res.results[c]["out"], dtype=np.float32)
    return out
